# revision 13
# baseline (speedup 1.0000x reference)
"""AMPBlock0 (BigVGAN) Trainium2 kernel — 8-core batch-parallel Bass/Tile.

Per core: one batch element [C=512, T=8192] streamed in T-chunks through the
fused chain  AFA1 -> conv1 -> AFA2 -> conv2 -> AFA3 -> +x  entirely in SBUF.

AFA (alias-free activation) = up2x (12-tap polyphase FIR) -> snakebeta -> down2x
(12-tap polyphase FIR).  snakebeta  s = y + c*sin^2(a*y)  is computed as
sin^2(pi*g) with g = w - rint(w), w = y*(a/pi): the ScalarE Sin LUT only
accepts [-pi, pi], so w is range-reduced with the fp32 magic-number trick
(v = w + 1.5*2^23 rounds to rint(w) + M in one fused multiply-add); squaring
makes the parity sign of sin(pi*(w - k)) irrelevant.

Layout: channels on partitions ([128, 4 blocks, W] tiles), time along free.
Dense 3-tap convs are 12-matmul PSUM accumulation chains on TensorE (bf16 in,
fp32 accumulate), evicted by ScalarE with fused bias add.  FIR taps are fused
multiply-adds (scalar_tensor_tensor) distributed over VectorE / GpSimd /
TensorE (diagonal-matmul form) per ENGINE_PLAN.
"""

import math

import numpy as np

try:
    import concourse.bass as bass
except ImportError:  # container staging path
    import sys

    sys.path.insert(0, "/opt/trn_rl_repo")
    import concourse.bass as bass

import ml_dtypes
import concourse.mybir as mybir
from concourse import bacc
from concourse.tile import TileContext
from concourse.bass_utils import run_bass_kernel_spmd

F32 = mybir.dt.float32
BF16 = mybir.dt.bfloat16
MULT = mybir.AluOpType.mult
ADD = mybir.AluOpType.add
SUB = mybir.AluOpType.subtract
SIN = mybir.ActivationFunctionType.Sin
IDENT = mybir.ActivationFunctionType.Identity
MAGIC = 12582912.0  # 1.5 * 2^23 — fp32 RNE-to-integer magic constant
PI = math.pi

C = 512
NB = 4  # channel blocks of 128
T = 8192
TC = 1024  # chunk interior
HALO = 32
HALF_PI = math.pi / 2.0
EPS = 1e-9

# Stage margins (local cols [m, W-m) computed), all even; valid = [HALO, W-HALO)
M_UP1, M_Z1, M_C1 = 4, 8, 10
M_UP2, M_Z2, M_C2 = 14, 18, 20
M_UP3, M_Z3 = 24, 28


def _kaiser_sinc_filter1d(cutoff, half_width, kernel_size):
    even = kernel_size % 2 == 0
    half_size = kernel_size // 2
    delta_f = 4.0 * half_width
    A = 2.285 * (half_size - 1) * np.pi * delta_f + 7.95
    if A > 50.0:
        beta = 0.1102 * (A - 8.7)
    elif A >= 21.0:
        beta = 0.5842 * (A - 21.0) ** 0.4 + 0.07886 * (A - 21.0)
    else:
        beta = 0.0
    window = np.kaiser(kernel_size, beta)
    if even:
        time = np.arange(-half_size, half_size) + 0.5
    else:
        time = np.arange(kernel_size) - half_size
    f = 2.0 * cutoff * window * np.sinc(2.0 * cutoff * time)
    f = f / f.sum()
    return f.astype(np.float32)


_F = _kaiser_sinc_filter1d(0.25, 0.3, 12)
# up phases (x2 interpolation gain folded in):  ye[t]=sum_a FE[a]*x[t+a-3],
# yo[t]=sum_a FO[a]*x[t+a-2]
FE = [float(2.0 * _F[2 * a]) for a in range(6)]
FO = [float(2.0 * _F[2 * a + 1]) for a in range(6)]
# down: z[t] = sum_b GO[b]*so[t+b-3] + GE[b]*se[t+b-2]
GO = [float(_F[2 * b]) for b in range(6)]
GE = [float(_F[2 * b + 1]) for b in range(6)]

# Which engine runs each FIR site: ('dve'|'gps'|'pe') per (site, block).
# Tuned on hardware; default all-DVE is correct everywhere.
ENGINE_PLAN = {
    "u1": ["dve"] * NB,
    "d1": ["dve"] * NB,
    "u2": ["dve"] * NB,
    "d2": ["dve"] * NB,
    "u3": ["dve"] * NB,
    "d3": ["dve"] * NB,
}


def build(T_=T, tc_=TC, engine_plan=None, num_devices=8):
    """Build the per-core Bass module (input [C, T_] f32 + bf16 copy -> out)."""
    ep = engine_plan or ENGINE_PLAN
    W = tc_ + 2 * HALO
    nchunk = T_ // tc_
    assert T_ % tc_ == 0

    nc = bacc.Bacc(
        "TRN2", target_bir_lowering=False, debug=False, num_devices=num_devices
    )
    xf_h = nc.declare_dram_parameter("x", [C, T_], F32, isOutput=False)
    xbf_h = nc.declare_dram_parameter("xbf", [C, T_], BF16, isOutput=False)
    w1_h = nc.declare_dram_parameter("w1p", [C, 3, C], BF16, isOutput=False)
    w2_h = nc.declare_dram_parameter("w2p", [C, 3, C], BF16, isOutput=False)
    # pv rows: 0:a1/pi  1:c1  2:a2/pi  3:c2  4:b1  5:b2
    pv_h = nc.declare_dram_parameter("pv", [6, C], F32, isOutput=False)
    fd_h = nc.declare_dram_parameter("fdiag", [24, 128, 128], BF16, isOutput=False)
    out_h = nc.declare_dram_parameter("out", [C, T_], F32, isOutput=True)

    xf_r = xf_h.rearrange("(b p) t -> p b t", p=128)
    xbf_r = xbf_h.rearrange("(b p) t -> p b t", p=128)
    w1_r = w1_h.rearrange("(b p) d co -> p b (d co)", p=128)
    w2_r = w2_h.rearrange("(b p) d co -> p b (d co)", p=128)
    pv_r = pv_h.rearrange("r (b p) -> p r b", p=128)
    fd_r = fd_h.rearrange("k p m -> p k m")
    out_r = out_h.rearrange("(b p) t -> p b t", p=128)

    any_pe = any(e == "pe" for site in ep.values() for e in site)

    with TileContext(nc) as tc:
        with (
            tc.tile_pool(name="const", bufs=1) as cpool,
            tc.tile_pool(name="io", bufs=2) as iopool,
            tc.tile_pool(name="work", bufs=2) as wpool,
            tc.tile_pool(name="psum", bufs=4, space="PSUM") as ppool,
        ):
            w1sb = cpool.tile([128, NB, 3 * C], BF16, name="w1sb")
            nc.sync.dma_start(out=w1sb[:], in_=w1_r[:])
            w2sb = cpool.tile([128, NB, 3 * C], BF16, name="w2sb")
            nc.sync.dma_start(out=w2sb[:], in_=w2_r[:])
            pvsb = cpool.tile([128, 6, NB], F32, name="pvsb")
            nc.sync.dma_start(out=pvsb[:], in_=pv_r[:])
            if any_pe:
                fdsb = cpool.tile([128, 24, 128], BF16, name="fdsb")
                nc.sync.dma_start(out=fdsb[:], in_=fd_r[:])
            else:
                fdsb = None

            def fill_left(tile_, c1, c0):
                # fill cols [c0, c1) with the value at col c1 (log-doubling)
                rep_lo, have = c1, 1
                while rep_lo > c0:
                    w = min(have, rep_lo - c0)
                    nc.vector.tensor_copy(
                        tile_[:, :, rep_lo - w : rep_lo],
                        tile_[:, :, rep_lo : rep_lo + w],
                    )
                    rep_lo -= w
                    have += w

            def fill_right(tile_, c0, c1):
                # fill cols (c0, c1] ... i.e. [c0+1, c1+1) with value at col c0
                rep_hi, have = c0 + 1, 1
                while rep_hi < c1 + 1:
                    w = min(have, c1 + 1 - rep_hi)
                    nc.vector.tensor_copy(
                        tile_[:, :, rep_hi : rep_hi + w],
                        tile_[:, :, rep_hi - w : rep_hi],
                    )
                    rep_hi += w
                    have += w

            def fir_taps(dst, srcs_shifts_taps, m, n, eng, fd_base):
                """dst[:, b, m:m+n] = sum taps[j]*src[:, b, m+shift_j : ...].

                srcs_shifts_taps: list of (src_tile, shift, tap).
                eng: per-block engine list. 'pe' uses diagonal matmuls with
                fdiag[fd_base + j] and evicts via ScalarE.
                """
                dve_like = {"dve": nc.vector, "gps": nc.gpsimd}
                blocks_by_eng = {}
                for b in range(NB):
                    blocks_by_eng.setdefault(eng[b], []).append(b)
                for e, blks in blocks_by_eng.items():
                    if e in dve_like:
                        v = dve_like[e]
                        if blks == list(range(NB)):
                            sel = [(dst, slice(None))]
                        else:
                            sel = [(dst, b) for b in blks]
                        for d_, bsel in sel:
                            dv = d_[:, bsel, m : m + n]
                            first = True
                            for src, sh, tap in srcs_shifts_taps:
                                sv = src[:, bsel, m + sh : m + sh + n]
                                if first:
                                    v.tensor_scalar(
                                        out=dv, in0=sv, scalar1=tap, scalar2=None,
                                        op0=MULT,
                                    )
                                    first = False
                                else:
                                    v.scalar_tensor_tensor(
                                        out=dv, in0=sv, scalar=tap, in1=dv,
                                        op0=MULT, op1=ADD,
                                    )
                    elif e == "pe":
                        for b in blks:
                            for o in range(0, n, 512):
                                sw = min(512, n - o)
                                ps = ppool.tile([128, sw], F32, tag="fps", bufs=4)
                                for j, (src, sh, tap) in enumerate(srcs_shifts_taps):
                                    nc.tensor.matmul(
                                        ps[:],
                                        fdsb[:, fd_base + j, :],
                                        src[:, b, m + sh + o : m + sh + o + sw],
                                        start=(j == 0),
                                        stop=(j == len(srcs_shifts_taps) - 1),
                                    )
                                nc.scalar.activation(
                                    out=dst[:, b, m + o : m + o + sw], in_=ps[:],
                                    func=IDENT, scale=1.0,
                                )
                    else:
                        raise ValueError(e)

            def fir_up(dst_e, dst_o, src, m, eng, fd_base=0):
                n = W - 2 * m
                fir_taps(dst_e, [(src, a - 3, FE[a]) for a in range(6)], m, n, eng,
                         fd_base)
                fir_taps(dst_o, [(src, a - 2, FO[a]) for a in range(6)], m, n, eng,
                         fd_base + 6)

            def fir_down(dst, se, so, m, eng):
                n = W - 2 * m
                taps = [(so, b - 3, GO[b]) for b in range(6)] + [
                    (se, b - 2, GE[b]) for b in range(6)
                ]
                fir_taps(dst, taps, m, n, eng, 12)

            def snake(ye, yo, m, r_a, r_c):
                # s = y + c*sin^2(pi*g),  g = w - rint(w),  w = y*(a/pi)
                n = W - 2 * m
                for yp in (ye, yo):
                    vt = wpool.tile([128, NB, W], F32, tag="vt", bufs=1, name="vt")
                    t2 = wpool.tile([128, NB, W], BF16, tag="t2", bufs=1, name="t2")
                    g = wpool.tile([128, NB, W], BF16, tag="tcos", bufs=2, name="g")
                    for b in range(NB):
                        # v = rint(w) + M  (fused multiply-add, fp32 RNE)
                        nc.vector.tensor_scalar(
                            out=vt[:, b, m : m + n], in0=yp[:, b, m : m + n],
                            scalar1=pvsb[:, r_a, b : b + 1], scalar2=MAGIC,
                            op0=MULT, op1=ADD,
                        )
                    # t2 = rint(w)  (exact in bf16 for |w| < 64)
                    nc.vector.tensor_scalar(
                        out=t2[:, :, m : m + n], in0=vt[:, :, m : m + n],
                        scalar1=MAGIC, scalar2=None, op0=SUB,
                    )
                    for b in range(NB):
                        # g = y*(a/pi) - rint(w)
                        nc.vector.scalar_tensor_tensor(
                            out=g[:, b, m : m + n], in0=yp[:, b, m : m + n],
                            scalar=pvsb[:, r_a, b : b + 1],
                            in1=t2[:, b, m : m + n], op0=MULT, op1=SUB,
                        )
                    # sn = sin(pi*g), in-place over g
                    nc.scalar.activation(
                        out=g[:, :, m : m + n], in_=g[:, :, m : m + n],
                        func=SIN, scale=PI,
                    )
                    # sq = sn*sn, in-place over g
                    nc.gpsimd.tensor_tensor(
                        out=g[:, :, m : m + n], in0=g[:, :, m : m + n],
                        in1=g[:, :, m : m + n], op=MULT,
                    )
                    for b in range(NB):
                        # s = sq*c + y, in-place over y
                        nc.vector.scalar_tensor_tensor(
                            out=yp[:, b, m : m + n], in0=g[:, b, m : m + n],
                            scalar=pvsb[:, r_c, b : b + 1],
                            in1=yp[:, b, m : m + n], op0=MULT, op1=ADD,
                        )

            def s_clamp(ye, yo, first, last):
                # down reads so[t-3..t+2], se[t-2..t+3]; 2T-grid edge clamping
                if first:  # s[u<0] -> se[HALO]
                    for c in (HALO - 3, HALO - 2, HALO - 1):
                        nc.vector.tensor_copy(
                            yo[:, :, c : c + 1], ye[:, :, HALO : HALO + 1]
                        )
                    for c in (HALO - 2, HALO - 1):
                        nc.vector.tensor_copy(
                            ye[:, :, c : c + 1], ye[:, :, HALO : HALO + 1]
                        )
                if last:  # s[u>2T-1] -> so[W-HALO-1]
                    e = W - HALO - 1
                    for c in (e + 1, e + 2, e + 3):
                        nc.vector.tensor_copy(
                            ye[:, :, c : c + 1], yo[:, :, e : e + 1]
                        )
                        nc.vector.tensor_copy(
                            yo[:, :, c : c + 1], yo[:, :, e : e + 1]
                        )

            def conv(dst, src, wsb, r_bias, m):
                n = W - 2 * m
                for co in range(NB):
                    for o in range(0, n, 512):
                        sw = min(512, n - o)
                        ps = ppool.tile([128, sw], F32, tag="cps", bufs=4,
                                        name="cps")
                        k = 0
                        for d in range(3):
                            for cb in range(NB):
                                nc.tensor.matmul(
                                    ps[:],
                                    wsb[:, cb, d * C + co * 128 : d * C + (co + 1) * 128],
                                    src[:, cb, m + o + d - 1 : m + o + d - 1 + sw],
                                    start=(k == 0),
                                    stop=(k == 11),
                                )
                                k += 1
                        nc.scalar.activation(
                            out=dst[:, co, m + o : m + o + sw], in_=ps[:],
                            func=IDENT, bias=pvsb[:, r_bias, co : co + 1], scale=1.0,
                        )

            for ci in range(nchunk):
                t0 = ci * tc_
                first = ci == 0
                last = ci == nchunk - 1
                lo, hi = t0 - HALO, t0 + tc_ + HALO
                slo, shi = max(lo, 0), min(hi, T_)
                d0 = slo - lo

                xbf = iopool.tile([128, NB, W], BF16, tag="xbf", name="xbf")
                nc.sync.dma_start(
                    out=xbf[:, :, d0 : d0 + shi - slo], in_=xbf_r[:, :, slo:shi]
                )
                if first:
                    fill_left(xbf, HALO, 0)
                if last:
                    fill_right(xbf, W - HALO - 1, W - 1)

                # ---- AFA1 ----
                ye = wpool.tile([128, NB, W], BF16, tag="ye", bufs=3, name="ye")
                yo = wpool.tile([128, NB, W], BF16, tag="yo", bufs=3, name="yo")
                fir_up(ye, yo, xbf, M_UP1, ep["u1"])
                snake(ye, yo, M_UP1, 0, 1)
                s_clamp(ye, yo, first, last)
                z = wpool.tile([128, NB, W], BF16, tag="z", bufs=2, name="z")
                fir_down(z, ye, yo, M_Z1, ep["d1"])
                if first:
                    nc.vector.memset(z[:, :, HALO - 1 : HALO], 0.0)
                if last:
                    nc.vector.memset(z[:, :, W - HALO : W - HALO + 1], 0.0)
                zc = wpool.tile([128, NB, W], BF16, tag="zc", bufs=2, name="zc")
                conv(zc, z, w1sb, 4, M_C1)
                if first:
                    fill_left(zc, HALO, HALO - 3)
                if last:
                    fill_right(zc, W - HALO - 1, W - HALO + 2)

                # ---- AFA2 ----
                ye2 = wpool.tile([128, NB, W], BF16, tag="ye", bufs=3, name="ye2")
                yo2 = wpool.tile([128, NB, W], BF16, tag="yo", bufs=3, name="yo2")
                fir_up(ye2, yo2, zc, M_UP2, ep["u2"])
                snake(ye2, yo2, M_UP2, 0, 1)
                s_clamp(ye2, yo2, first, last)
                z2 = wpool.tile([128, NB, W], BF16, tag="z", bufs=2, name="z2")
                fir_down(z2, ye2, yo2, M_Z2, ep["d2"])
                if first:
                    nc.vector.memset(z2[:, :, HALO - 1 : HALO], 0.0)
                if last:
                    nc.vector.memset(z2[:, :, W - HALO : W - HALO + 1], 0.0)
                zc2 = wpool.tile([128, NB, W], BF16, tag="zc", bufs=2, name="zc2")
                conv(zc2, z2, w2sb, 5, M_C2)
                if first:
                    fill_left(zc2, HALO, HALO - 3)
                if last:
                    fill_right(zc2, W - HALO - 1, W - HALO + 2)

                # ---- AFA3 ----
                ye3 = wpool.tile([128, NB, W], BF16, tag="ye", bufs=3, name="ye3")
                yo3 = wpool.tile([128, NB, W], BF16, tag="yo", bufs=3, name="yo3")
                fir_up(ye3, yo3, zc2, M_UP3, ep["u3"])
                snake(ye3, yo3, M_UP3, 2, 3)
                s_clamp(ye3, yo3, first, last)
                z3 = wpool.tile([128, NB, W], BF16, tag="z", bufs=2, name="z3")
                fir_down(z3, ye3, yo3, M_Z3, ep["d3"])

                # ---- residual ----
                xf = iopool.tile([128, NB, tc_], F32, tag="xf", name="xf")
                nc.sync.dma_start(out=xf[:], in_=xf_r[:, :, t0 : t0 + tc_])
                nc.vector.tensor_tensor(
                    out=xf[:], in0=z3[:, :, HALO : HALO + tc_], in1=xf[:], op=ADD
                )
                nc.sync.dma_start(out=out_r[:, :, t0 : t0 + tc_], in_=xf[:])

    nc.compile()
    return nc


def _host_inputs(x, w1, b1, w2, b2, alpha1, beta1, alpha2, beta2):
    a1 = np.exp(np.asarray(alpha1, np.float64))
    c1 = 1.0 / (np.exp(np.asarray(beta1, np.float64)) + EPS)
    a2 = np.exp(np.asarray(alpha2, np.float64))
    c2 = 1.0 / (np.exp(np.asarray(beta2, np.float64)) + EPS)
    pv = np.stack([a1 / PI, c1, a2 / PI, c2,
                   np.asarray(b1, np.float64), np.asarray(b2, np.float64)])
    pv = np.ascontiguousarray(pv).astype(np.float32)
    w1p = np.ascontiguousarray(np.asarray(w1).transpose(1, 2, 0)).astype(
        ml_dtypes.bfloat16
    )
    w2p = np.ascontiguousarray(np.asarray(w2).transpose(1, 2, 0)).astype(
        ml_dtypes.bfloat16
    )
    # diag filter tiles for the PE tap path: 6 FE, 6 FO, 6 GO, 6 GE
    eye = np.eye(128, dtype=np.float32)
    taps24 = FE + FO + GO + GE
    fdiag = np.ascontiguousarray(
        np.stack([t * eye for t in taps24])
    ).astype(ml_dtypes.bfloat16)
    xbf = np.asarray(x).astype(ml_dtypes.bfloat16)
    return pv, w1p, w2p, fdiag, xbf


_compiled = {}


def _get_compiled():
    if "nc" not in _compiled:
        _compiled["nc"] = build()
    return _compiled["nc"]


def make_in_maps(x, w1, b1, w2, b2, alpha1, beta1, alpha2, beta2):
    pv, w1p, w2p, fdiag, xbf = _host_inputs(
        x, w1, b1, w2, b2, alpha1, beta1, alpha2, beta2
    )
    B = x.shape[0]
    return [
        {
            "x": np.ascontiguousarray(np.asarray(x[i], np.float32)),
            "xbf": np.ascontiguousarray(xbf[i]),
            "w1p": w1p,
            "w2p": w2p,
            "pv": pv,
            "fdiag": fdiag,
        }
        for i in range(B)
    ]


def kernel(x, w1, b1, w2, b2, alpha1, beta1, alpha2, beta2):
    x = np.asarray(x)
    B = x.shape[0]
    assert x.shape == (B, C, T) and B == 8, x.shape
    nc = _get_compiled()
    in_maps = make_in_maps(x, w1, b1, w2, b2, alpha1, beta1, alpha2, beta2)
    res = run_bass_kernel_spmd(nc, in_maps, core_ids=list(range(B)))
    return np.stack([res.results[i]["out"] for i in range(B)], axis=0)


# revision 34
# speedup vs baseline: 4042.6433x; 4042.6433x over previous
"""AMPBlock0 (BigVGAN) Trainium2 kernel — 8-core batch-parallel Bass/Tile.

Per core: one batch element [C=512, T=8192] streamed in T-chunks through the
fused chain  AFA1 -> conv1 -> AFA2 -> conv2 -> AFA3 -> +x  entirely in SBUF.

AFA (alias-free activation) = up2x (12-tap polyphase FIR) -> snakebeta -> down2x
(12-tap polyphase FIR).  snakebeta  s = y + c*sin^2(a*y)  is computed as
sin^2(pi*g) with g = w - rint(w), w = y*(a/pi): the ScalarE Sin LUT only
accepts [-pi, pi], so w is range-reduced with the fp32 magic-number trick
(v = w + 1.5*2^23 rounds to rint(w) + M in one fused multiply-add); squaring
makes the parity sign of sin(pi*(w - k)) irrelevant.

Layout: channels on partitions ([128, 4 blocks, W] tiles), time along free.
Dense 3-tap convs are 12-matmul PSUM accumulation chains on TensorE (bf16 in,
fp32 accumulate), evicted by ScalarE with fused bias add.  FIR taps are fused
multiply-adds (scalar_tensor_tensor) distributed over VectorE / GpSimd /
TensorE (diagonal-matmul form) per ENGINE_PLAN.
"""

import math

import numpy as np

try:
    import concourse.bass as bass
except ImportError:  # container staging path
    import sys

    sys.path.insert(0, "/opt/trn_rl_repo")
    import concourse.bass as bass

import ml_dtypes
import concourse.mybir as mybir
from concourse import bacc
from concourse.tile import TileContext
from concourse.bass_utils import run_bass_kernel_spmd

F32 = mybir.dt.float32
BF16 = mybir.dt.bfloat16
MULT = mybir.AluOpType.mult
ADD = mybir.AluOpType.add
SUB = mybir.AluOpType.subtract
SIN = mybir.ActivationFunctionType.Sin
IDENT = mybir.ActivationFunctionType.Identity
MAGIC = 12582912.0  # 1.5 * 2^23 — fp32 RNE-to-integer magic constant
PI = math.pi

C = 512
NB = 4  # channel blocks of 128
T = 8192
TC = 512  # chunk interior
HALO = 20
HALF_PI = math.pi / 2.0
EPS = 1e-9

# Stage margins (local cols [m, W-m) computed); valid = [HALO, W-HALO)
M_UP1, M_Z1, M_C1 = 3, 6, 7
M_UP2, M_Z2, M_C2 = 10, 13, 14
M_UP3, M_Z3 = 17, 20


def _kaiser_sinc_filter1d(cutoff, half_width, kernel_size):
    even = kernel_size % 2 == 0
    half_size = kernel_size // 2
    delta_f = 4.0 * half_width
    A = 2.285 * (half_size - 1) * np.pi * delta_f + 7.95
    if A > 50.0:
        beta = 0.1102 * (A - 8.7)
    elif A >= 21.0:
        beta = 0.5842 * (A - 21.0) ** 0.4 + 0.07886 * (A - 21.0)
    else:
        beta = 0.0
    window = np.kaiser(kernel_size, beta)
    if even:
        time = np.arange(-half_size, half_size) + 0.5
    else:
        time = np.arange(kernel_size) - half_size
    f = 2.0 * cutoff * window * np.sinc(2.0 * cutoff * time)
    f = f / f.sum()
    return f.astype(np.float32)


_F = _kaiser_sinc_filter1d(0.25, 0.3, 12)
# up phases (x2 interpolation gain folded in):  ye[t]=sum_a FE[a]*x[t+a-3],
# yo[t]=sum_a FO[a]*x[t+a-2]
FE = [float(2.0 * _F[2 * a]) for a in range(6)]
FO = [float(2.0 * _F[2 * a + 1]) for a in range(6)]
# down: z[t] = sum_b GO[b]*so[t+b-3] + GE[b]*se[t+b-2]
GO = [float(_F[2 * b]) for b in range(6)]
GE = [float(_F[2 * b + 1]) for b in range(6)]

# Which engine runs each FIR site: ('dve'|'pe') per (site, block).
# 'pe' = diagonal-matmul accumulation in PSUM + ScalarE eviction.
# Tuned on hardware; all-DVE is correct everywhere.
ENGINE_PLAN = {
    "u1": ["pe"] * NB,
    "d1": ["pe", "pe", "dve", "dve"],
    "u2": ["pe"] * NB,
    "d2": ["pe", "pe", "dve", "dve"],
    "u3": ["pe"] * NB,
    "d3": ["pe"] * NB,
}
# +1-shifted shadow copies for DVE odd-shift taps: helps real silicon
# (alignment-gated 2x mode), pure overhead under the scheduler cost model.
USE_SHADOWS = False
# DVE tap form: 'stt' (1 fused op, 1x rate) or 'pair' (TS mult + TT add,
# both perf-mode eligible).
DVE_TAP_FORM = "pair"
GROUP = 5
SQ_ENGINE = "dve"   # "gps" | "dve"
MERGE_FORM = "stt"  # "gps_tt" | "dve_tt" | "stt"
BUFS = {"ye": 6, "yo": 6, "z": 5, "zc": 5, "tcos": 4, "xbf": 4, "xf": 2}


def build(T_=T, tc_=TC, engine_plan=None, num_devices=8):
    """Build the per-core Bass module (input [C, T_] f32 + bf16 copy -> out)."""
    ep = engine_plan or ENGINE_PLAN
    W = tc_ + 2 * HALO
    nchunk = T_ // tc_
    assert T_ % tc_ == 0

    nc = bacc.Bacc(
        "TRN2", target_bir_lowering=False, debug=False, num_devices=num_devices
    )
    xf_h = nc.declare_dram_parameter("x", [C, T_], F32, isOutput=False)
    xbf_h = nc.declare_dram_parameter("xbf", [C, T_], BF16, isOutput=False)
    w1_h = nc.declare_dram_parameter("w1p", [C, 3, C], BF16, isOutput=False)
    w2_h = nc.declare_dram_parameter("w2p", [C, 3, C], BF16, isOutput=False)
    # pv rows: 0:a1/pi  1:c1  2:a2/pi  3:c2  4:b1  5:b2
    pv_h = nc.declare_dram_parameter("pv", [6, C], F32, isOutput=False)
    fd_h = nc.declare_dram_parameter("fdiag", [24, 128, 128], BF16, isOutput=False)
    out_h = nc.declare_dram_parameter("out", [C, T_], F32, isOutput=True)

    xf_r = xf_h.rearrange("(b p) t -> p b t", p=128)
    xbf_r = xbf_h.rearrange("(b p) t -> p b t", p=128)
    w1_r = w1_h.rearrange("(b p) d co -> p b (d co)", p=128)
    w2_r = w2_h.rearrange("(b p) d co -> p b (d co)", p=128)
    pv_r = pv_h.rearrange("r (b p) -> p r b", p=128)
    fd_r = fd_h.rearrange("k p m -> p k m")
    out_r = out_h.rearrange("(b p) t -> p b t", p=128)

    any_pe = any(e == "pe" for site in ep.values() for e in site)

    with TileContext(nc) as tc:
        with (
            tc.tile_pool(name="const", bufs=1) as cpool,
            tc.tile_pool(name="io", bufs=2) as iopool,
            tc.tile_pool(name="work", bufs=2) as wpool,
            tc.tile_pool(name="psum", bufs=4, space="PSUM") as ppool,
        ):
            w1sb = cpool.tile([128, NB, 3 * C], BF16, name="w1sb")
            nc.sync.dma_start(out=w1sb[:], in_=w1_r[:])
            w2sb = cpool.tile([128, NB, 3 * C], BF16, name="w2sb")
            nc.sync.dma_start(out=w2sb[:], in_=w2_r[:])
            pvsb = cpool.tile([128, 6, NB], F32, name="pvsb")
            nc.sync.dma_start(out=pvsb[:], in_=pv_r[:])
            negm = cpool.tile([128, 1], F32, name="negm")
            nc.vector.memset(negm[:], -MAGIC)
            if any_pe:
                fdsb = cpool.tile([128, 24, 128], BF16, name="fdsb")
                nc.sync.dma_start(out=fdsb[:], in_=fd_r[:])
            else:
                fdsb = None

            def fill_left(tile_, c1, c0):
                # fill cols [c0, c1) with the value at col c1 (log-doubling)
                rep_lo, have = c1, 1
                while rep_lo > c0:
                    w = min(have, rep_lo - c0)
                    nc.vector.tensor_copy(
                        tile_[:, :, rep_lo - w : rep_lo],
                        tile_[:, :, rep_lo : rep_lo + w],
                    )
                    rep_lo -= w
                    have += w

            def fill_right(tile_, c0, c1):
                # fill cols (c0, c1] ... i.e. [c0+1, c1+1) with value at col c0
                rep_hi, have = c0 + 1, 1
                while rep_hi < c1 + 1:
                    w = min(have, c1 + 1 - rep_hi)
                    nc.vector.tensor_copy(
                        tile_[:, :, rep_hi : rep_hi + w],
                        tile_[:, :, rep_hi - w : rep_hi],
                    )
                    rep_hi += w
                    have += w

            def mk_shadow(src, tag, lo, hi):
                """+1-shifted copy (sh[c] = src[c+1]) over src-read range
                [lo, hi) so odd-shift taps read 4B-aligned bf16.  On ScalarE:
                alignment-insensitive, spare capacity."""
                sh = wpool.tile([128, NB, W], BF16, tag=tag, bufs=1, name=tag)
                nc.scalar.activation(
                    out=sh[:, :, lo - 1 : hi - 1], in_=src[:, :, lo:hi],
                    func=IDENT, scale=1.0,
                )
                return sh

            def fir_taps(dst, srcs_shifts_taps, m, n, eng, fd_base, shadows=None):
                """dst[:, b, m:m+n] = sum taps[j]*src[:, b, m+shift_j : ...].

                srcs_shifts_taps: list of (src_tile, shift, tap).
                eng: per-block engine list. 'pe' uses diagonal matmuls with
                fdiag[fd_base + j] and evicts via ScalarE.  shadows: id(src)
                -> +1-shifted copy, used by DVE blocks for odd shifts.
                """
                shadows = shadows or {}
                blocks_by_eng = {}
                for b in range(NB):
                    blocks_by_eng.setdefault(eng[b], []).append(b)
                for e, blks in blocks_by_eng.items():
                    if e == "dve":
                        if blks == list(range(NB)):
                            sel = [slice(None)]
                        else:
                            sel = blks
                        for bsel in sel:
                            dv = dst[:, bsel, m : m + n]
                            first = True
                            for src, sh, tap in srcs_shifts_taps:
                                off = m + sh
                                if off % 2 and id(src) in shadows:
                                    src = shadows[id(src)]
                                    off -= 1
                                sv = src[:, bsel, off : off + n]
                                if first:
                                    nc.vector.tensor_scalar(
                                        out=dv, in0=sv, scalar1=tap, scalar2=None,
                                        op0=MULT,
                                    )
                                    first = False
                                elif DVE_TAP_FORM == "pair":
                                    tmp = wpool.tile(
                                        [128, NB, W], BF16, tag="ttmp", bufs=2,
                                        name="ttmp",
                                    )
                                    tv = tmp[:, bsel, m : m + n]
                                    nc.vector.tensor_scalar(
                                        out=tv, in0=sv, scalar1=tap, scalar2=None,
                                        op0=MULT,
                                    )
                                    nc.vector.tensor_tensor(
                                        out=dv, in0=tv, in1=dv, op=ADD
                                    )
                                else:
                                    nc.vector.scalar_tensor_tensor(
                                        out=dv, in0=sv, scalar=tap, in1=dv,
                                        op0=MULT, op1=ADD,
                                    )
                    elif e == "pe":
                        nt = len(srcs_shifts_taps)
                        for b in blks:
                            ps = ppool.tile([128, n], F32, tag="ps", bufs=4)
                            for o in range(0, n, 512):
                                sw = min(512, n - o)
                                for j, (src, sh, tap) in enumerate(srcs_shifts_taps):
                                    nc.tensor.matmul(
                                        ps[:, o : o + sw],
                                        fdsb[:, fd_base + j, :],
                                        src[:, b, m + sh + o : m + sh + o + sw],
                                        start=(j == 0),
                                        stop=(j == nt - 1),
                                    )
                            nc.scalar.activation(
                                out=dst[:, b, m : m + n], in_=ps[:],
                                func=IDENT, scale=1.0,
                            )
                    else:
                        raise ValueError(e)

            def fir_up(dst_e, dst_o, src, m, eng, fd_base=0):
                n = W - 2 * m
                shadows = {}
                if USE_SHADOWS and any(e == "dve" for e in eng):
                    shadows[id(src)] = mk_shadow(src, "shu", m - 3, W - m + 3)
                fir_taps(dst_e, [(src, a - 3, FE[a]) for a in range(6)], m, n, eng,
                         fd_base, shadows)
                fir_taps(dst_o, [(src, a - 2, FO[a]) for a in range(6)], m, n, eng,
                         fd_base + 6, shadows)

            def fir_down(dst, se, so, m, eng):
                n = W - 2 * m
                shadows = {}
                if USE_SHADOWS and any(e == "dve" for e in eng):
                    shadows[id(se)] = mk_shadow(se, "she", m - 3, W - m + 3)
                    shadows[id(so)] = mk_shadow(so, "sho", m - 3, W - m + 3)
                taps = [(so, b - 3, GO[b]) for b in range(6)] + [
                    (se, b - 2, GE[b]) for b in range(6)
                ]
                fir_taps(dst, taps, m, n, eng, 12, shadows)

            def snake(ye, yo, m, r_a, r_c):
                # s = y + c*sin^2(pi*g),  g = w - rint(w),  w = y*(a/pi)
                n = W - 2 * m
                for yp in (ye, yo):
                    g = wpool.tile([128, NB, W], BF16, tag="tcos", bufs=BUFS["tcos"], name="g")
                    for b in range(NB):
                        vt = wpool.tile([128, W], F32, tag="vt", bufs=4, name="vt")
                        t2 = wpool.tile([128, W], BF16, tag="t2", bufs=4, name="t2")
                        # v = rint(w) + M  (fused multiply-add, fp32 RNE)
                        nc.vector.tensor_scalar(
                            out=vt[:, m : m + n], in0=yp[:, b, m : m + n],
                            scalar1=pvsb[:, r_a, b : b + 1], scalar2=MAGIC,
                            op0=MULT, op1=ADD,
                        )
                        # t2 = rint(w) = v - M  (exact in bf16 for |w| < 64)
                        nc.scalar.activation(
                            out=t2[:, m : m + n], in_=vt[:, m : m + n],
                            func=IDENT, bias=negm[:, 0:1], scale=1.0,
                        )
                        # g = y*(a/pi) - rint(w)
                        nc.vector.scalar_tensor_tensor(
                            out=g[:, b, m : m + n], in0=yp[:, b, m : m + n],
                            scalar=pvsb[:, r_a, b : b + 1],
                            in1=t2[:, m : m + n], op0=MULT, op1=SUB,
                        )
                    # sn = sin(pi*g), in-place over g
                    nc.scalar.activation(
                        out=g[:, :, m : m + n], in_=g[:, :, m : m + n],
                        func=SIN, scale=PI,
                    )
                    # sq = sn*sn, in-place over g
                    sq_eng = nc.gpsimd if SQ_ENGINE == "gps" else nc.vector
                    sq_eng.tensor_tensor(
                        out=g[:, :, m : m + n], in0=g[:, :, m : m + n],
                        in1=g[:, :, m : m + n], op=MULT,
                    )
                    if MERGE_FORM == "stt":
                        for b in range(NB):
                            nc.vector.scalar_tensor_tensor(
                                out=yp[:, b, m : m + n], in0=g[:, b, m : m + n],
                                scalar=pvsb[:, r_c, b : b + 1],
                                in1=yp[:, b, m : m + n], op0=MULT, op1=ADD,
                            )
                    else:
                        for b in range(NB):
                            # q = sq*c (4x-eligible tensor_scalar)
                            nc.vector.tensor_scalar(
                                out=g[:, b, m : m + n], in0=g[:, b, m : m + n],
                                scalar1=pvsb[:, r_c, b : b + 1], scalar2=None,
                                op0=MULT,
                            )
                        # s = q + y, in-place over y
                        m_eng = nc.gpsimd if MERGE_FORM == "gps_tt" else nc.vector
                        m_eng.tensor_tensor(
                            out=yp[:, :, m : m + n], in0=g[:, :, m : m + n],
                            in1=yp[:, :, m : m + n], op=ADD,
                        )

            def s_clamp(ye, yo, first, last):
                # down reads so[t-3..t+2], se[t-2..t+3]; 2T-grid edge clamping
                if first:  # s[u<0] -> se[HALO]
                    for c in (HALO - 3, HALO - 2, HALO - 1):
                        nc.vector.tensor_copy(
                            yo[:, :, c : c + 1], ye[:, :, HALO : HALO + 1]
                        )
                    for c in (HALO - 2, HALO - 1):
                        nc.vector.tensor_copy(
                            ye[:, :, c : c + 1], ye[:, :, HALO : HALO + 1]
                        )
                if last:  # s[u>2T-1] -> so[W-HALO-1]
                    e = W - HALO - 1
                    for c in (e + 1, e + 2, e + 3):
                        nc.vector.tensor_copy(
                            ye[:, :, c : c + 1], yo[:, :, e : e + 1]
                        )
                        nc.vector.tensor_copy(
                            yo[:, :, c : c + 1], yo[:, :, e : e + 1]
                        )

            def conv(dst, src, wsb, r_bias, m):
                n = W - 2 * m
                for co in range(NB):
                    ps = ppool.tile([128, n], F32, tag="ps", bufs=4, name="cps")
                    for o in range(0, n, 512):
                        sw = min(512, n - o)
                        k = 0
                        for d in range(3):
                            for cb in range(NB):
                                nc.tensor.matmul(
                                    ps[:, o : o + sw],
                                    wsb[:, cb, d * C + co * 128 : d * C + (co + 1) * 128],
                                    src[:, cb, m + o + d - 1 : m + o + d - 1 + sw],
                                    start=(k == 0),
                                    stop=(k == 11),
                                )
                                k += 1
                    nc.scalar.activation(
                        out=dst[:, co, m : m + n], in_=ps[:],
                        func=IDENT, bias=pvsb[:, r_bias, co : co + 1],
                        scale=1.0,
                    )

            # --- stage-major emission over groups of chunks: each engine
            # gets GROUP chunks of the same stage back-to-back, so e.g. PE
            # runs chunk i+1's FIR matmuls while DVE/ACT/GpSimd run chunk
            # i's snake chain. ---
            def st_load(s):
                ci = s["ci"]
                lo, hi = ci * tc_ - HALO, ci * tc_ + tc_ + HALO
                slo, shi = max(lo, 0), min(hi, T_)
                d0 = slo - lo
                xbf = iopool.tile([128, NB, W], BF16, tag="xbf", bufs=BUFS["xbf"],
                                  name="xbf")
                nc.sync.dma_start(
                    out=xbf[:, :, d0 : d0 + shi - slo], in_=xbf_r[:, :, slo:shi]
                )
                if s["first"]:
                    fill_left(xbf, HALO, 0)
                if s["last"]:
                    fill_right(xbf, W - HALO - 1, W - 1)
                s["xbf"] = xbf

            def st_afa(k):
                up_m, z_m = ((M_UP1, M_Z1), (M_UP2, M_Z2), (M_UP3, M_Z3))[k]
                r_a, r_c = (0, 1) if k < 2 else (2, 3)

                def up(s):
                    src = s["xbf"] if k == 0 else s["zc"]
                    ye = wpool.tile([128, NB, W], BF16, tag="ye", bufs=BUFS["ye"],
                                    name="ye")
                    yo = wpool.tile([128, NB, W], BF16, tag="yo", bufs=BUFS["yo"],
                                    name="yo")
                    fir_up(ye, yo, src, up_m, ep[("u1", "u2", "u3")[k]])
                    s["ye"], s["yo"] = ye, yo

                def snk(s):
                    snake(s["ye"], s["yo"], up_m, r_a, r_c)
                    s_clamp(s["ye"], s["yo"], s["first"], s["last"])

                def down(s):
                    z = wpool.tile([128, NB, W], BF16, tag="z", bufs=BUFS["z"],
                                   name="z")
                    fir_down(z, s["ye"], s["yo"], z_m, ep[("d1", "d2", "d3")[k]])
                    if k < 2:
                        if s["first"]:
                            nc.vector.memset(z[:, :, HALO - 1 : HALO], 0.0)
                        if s["last"]:
                            nc.vector.memset(
                                z[:, :, W - HALO : W - HALO + 1], 0.0
                            )
                    s["z"] = z

                return [up, snk, down]

            def st_conv(k):
                def run(s):
                    wsb, r_bias, m = ((w1sb, 4, M_C1), (w2sb, 5, M_C2))[k]
                    zc = wpool.tile([128, NB, W], BF16, tag="zc", bufs=BUFS["zc"],
                                    name="zc")
                    conv(zc, s["z"], wsb, r_bias, m)
                    if s["first"]:
                        fill_left(zc, HALO, HALO - 3)
                    if s["last"]:
                        fill_right(zc, W - HALO - 1, W - HALO + 2)
                    s["zc"] = zc

                return run

            def st_resid(s):
                ci = s["ci"]
                t0 = ci * tc_
                xf = iopool.tile([128, NB, tc_], F32, tag="xf", bufs=BUFS["xf"],
                                 name="xf")
                nc.sync.dma_start(out=xf[:], in_=xf_r[:, :, t0 : t0 + tc_])
                nc.gpsimd.tensor_tensor(
                    out=xf[:], in0=s["z"][:, :, HALO : HALO + tc_], in1=xf[:],
                    op=ADD,
                )
                nc.sync.dma_start(out=out_r[:, :, t0 : t0 + tc_], in_=xf[:])

            stages = (
                [st_load]
                + st_afa(0)
                + [st_conv(0)]
                + st_afa(1)
                + [st_conv(1)]
                + st_afa(2)
                + [st_resid]
            )
            for g0 in range(0, nchunk, GROUP):
                chunks = [
                    {"ci": ci, "first": ci == 0, "last": ci == nchunk - 1}
                    for ci in range(g0, min(g0 + GROUP, nchunk))
                ]
                for stage in stages:
                    for s in chunks:
                        stage(s)

    nc.compile()
    return nc
def _host_inputs(x, w1, b1, w2, b2, alpha1, beta1, alpha2, beta2):
    a1 = np.exp(np.asarray(alpha1, np.float64))
    c1 = 1.0 / (np.exp(np.asarray(beta1, np.float64)) + EPS)
    a2 = np.exp(np.asarray(alpha2, np.float64))
    c2 = 1.0 / (np.exp(np.asarray(beta2, np.float64)) + EPS)
    pv = np.stack([a1 / PI, c1, a2 / PI, c2,
                   np.asarray(b1, np.float64), np.asarray(b2, np.float64)])
    pv = np.ascontiguousarray(pv).astype(np.float32)
    w1p = np.ascontiguousarray(np.asarray(w1).transpose(1, 2, 0)).astype(
        ml_dtypes.bfloat16
    )
    w2p = np.ascontiguousarray(np.asarray(w2).transpose(1, 2, 0)).astype(
        ml_dtypes.bfloat16
    )
    # diag filter tiles for the PE tap path: 6 FE, 6 FO, 6 GO, 6 GE
    eye = np.eye(128, dtype=np.float32)
    taps24 = FE + FO + GO + GE
    fdiag = np.ascontiguousarray(
        np.stack([t * eye for t in taps24])
    ).astype(ml_dtypes.bfloat16)
    xbf = np.asarray(x).astype(ml_dtypes.bfloat16)
    return pv, w1p, w2p, fdiag, xbf


_compiled = {}


def _get_compiled():
    if "nc" not in _compiled:
        _compiled["nc"] = build()
    return _compiled["nc"]


def make_in_maps(x, w1, b1, w2, b2, alpha1, beta1, alpha2, beta2):
    pv, w1p, w2p, fdiag, xbf = _host_inputs(
        x, w1, b1, w2, b2, alpha1, beta1, alpha2, beta2
    )
    B = x.shape[0]
    return [
        {
            "x": np.ascontiguousarray(np.asarray(x[i], np.float32)),
            "xbf": np.ascontiguousarray(xbf[i]),
            "w1p": w1p,
            "w2p": w2p,
            "pv": pv,
            "fdiag": fdiag,
        }
        for i in range(B)
    ]


def kernel(x, w1, b1, w2, b2, alpha1, beta1, alpha2, beta2):
    x = np.asarray(x)
    B = x.shape[0]
    assert x.shape == (B, C, T) and B == 8, x.shape
    nc = _get_compiled()
    in_maps = make_in_maps(x, w1, b1, w2, b2, alpha1, beta1, alpha2, beta2)
    res = run_bass_kernel_spmd(nc, in_maps, core_ids=list(range(B)))
    return np.stack([res.results[i]["out"] for i in range(B)], axis=0)


# revision 35
# speedup vs baseline: 4060.3207x; 1.0044x over previous
"""AMPBlock0 (BigVGAN) Trainium2 kernel — 8-core batch-parallel Bass/Tile.

Per core: one batch element [C=512, T=8192] streamed in T-chunks through the
fused chain  AFA1 -> conv1 -> AFA2 -> conv2 -> AFA3 -> +x  entirely in SBUF.

AFA (alias-free activation) = up2x (12-tap polyphase FIR) -> snakebeta -> down2x
(12-tap polyphase FIR).  snakebeta  s = y + c*sin^2(a*y)  is computed as
sin^2(pi*g) with g = w - rint(w), w = y*(a/pi): the ScalarE Sin LUT only
accepts [-pi, pi], so w is range-reduced with the fp32 magic-number trick
(v = w + 1.5*2^23 rounds to rint(w) + M in one fused multiply-add); squaring
makes the parity sign of sin(pi*(w - k)) irrelevant.

Layout: channels on partitions ([128, 4 blocks, W] tiles), time along free.
Dense 3-tap convs are 12-matmul PSUM accumulation chains on TensorE (bf16 in,
fp32 accumulate, 2-bank PSUM tiles), evicted by ScalarE with fused bias add.
FIR taps run mostly on TensorE as diagonal-matmul PSUM accumulations (the
systolic array is only ~1% utilized by a diagonal lhsT, but at 512 cols/matmul
it still beats VectorE for this op mix); the remainder are TS-mult + TT-add
pairs on VectorE, per ENGINE_PLAN.  Emission is stage-major over GROUPs of
chunks so each engine sees several chunks of the same stage back-to-back
(TensorE runs chunk i+1's FIRs while VectorE/ScalarE run chunk i's snake).
"""

import math

import numpy as np

try:
    import concourse.bass as bass
except ImportError:  # container staging path
    import sys

    sys.path.insert(0, "/opt/trn_rl_repo")
    import concourse.bass as bass

import ml_dtypes
import concourse.mybir as mybir
from concourse import bacc
from concourse.tile import TileContext
from concourse.bass_utils import run_bass_kernel_spmd

F32 = mybir.dt.float32
BF16 = mybir.dt.bfloat16
MULT = mybir.AluOpType.mult
ADD = mybir.AluOpType.add
SUB = mybir.AluOpType.subtract
SIN = mybir.ActivationFunctionType.Sin
IDENT = mybir.ActivationFunctionType.Identity
MAGIC = 12582912.0  # 1.5 * 2^23 — fp32 RNE-to-integer magic constant
PI = math.pi

C = 512
NB = 4  # channel blocks of 128
T = 8192
TC = 512  # chunk interior
HALO = 20
HALF_PI = math.pi / 2.0
EPS = 1e-9

# Stage margins (local cols [m, W-m) computed); valid = [HALO, W-HALO)
M_UP1, M_Z1, M_C1 = 3, 6, 7
M_UP2, M_Z2, M_C2 = 10, 13, 14
M_UP3, M_Z3 = 17, 20


def _kaiser_sinc_filter1d(cutoff, half_width, kernel_size):
    even = kernel_size % 2 == 0
    half_size = kernel_size // 2
    delta_f = 4.0 * half_width
    A = 2.285 * (half_size - 1) * np.pi * delta_f + 7.95
    if A > 50.0:
        beta = 0.1102 * (A - 8.7)
    elif A >= 21.0:
        beta = 0.5842 * (A - 21.0) ** 0.4 + 0.07886 * (A - 21.0)
    else:
        beta = 0.0
    window = np.kaiser(kernel_size, beta)
    if even:
        time = np.arange(-half_size, half_size) + 0.5
    else:
        time = np.arange(kernel_size) - half_size
    f = 2.0 * cutoff * window * np.sinc(2.0 * cutoff * time)
    f = f / f.sum()
    return f.astype(np.float32)


_F = _kaiser_sinc_filter1d(0.25, 0.3, 12)
# up phases (x2 interpolation gain folded in):  ye[t]=sum_a FE[a]*x[t+a-3],
# yo[t]=sum_a FO[a]*x[t+a-2]
FE = [float(2.0 * _F[2 * a]) for a in range(6)]
FO = [float(2.0 * _F[2 * a + 1]) for a in range(6)]
# down: z[t] = sum_b GO[b]*so[t+b-3] + GE[b]*se[t+b-2]
GO = [float(_F[2 * b]) for b in range(6)]
GE = [float(_F[2 * b + 1]) for b in range(6)]

# Which engine runs each FIR site: ('dve'|'pe') per (site, block).
# 'pe' = diagonal-matmul accumulation in PSUM + ScalarE eviction.
# Tuned on hardware; all-DVE is correct everywhere.
ENGINE_PLAN = {
    "u1": ["pe"] * NB,
    "d1": ["pe", "pe", "dve", "dve"],
    "u2": ["pe"] * NB,
    "d2": ["pe", "pe", "dve", "dve"],
    "u3": ["pe"] * NB,
    "d3": ["pe"] * NB,
}
# +1-shifted shadow copies for DVE odd-shift taps: helps real silicon
# (alignment-gated 2x mode), pure overhead under the scheduler cost model.
USE_SHADOWS = False
# DVE tap form: 'stt' (1 fused op, 1x rate) or 'pair' (TS mult + TT add,
# both perf-mode eligible).
DVE_TAP_FORM = "pair"
GROUP = 4
SQ_ENGINE = "dve"   # "gps" | "dve"
MERGE_FORM = "stt"  # "gps_tt" | "dve_tt" | "stt"
BUFS = {"ye": 6, "yo": 6, "z": 5, "zc": 5, "tcos": 4, "xbf": 4, "xf": 2}


def build(T_=T, tc_=TC, engine_plan=None, num_devices=8):
    """Build the per-core Bass module (input [C, T_] f32 + bf16 copy -> out)."""
    ep = engine_plan or ENGINE_PLAN
    W = tc_ + 2 * HALO
    nchunk = T_ // tc_
    assert T_ % tc_ == 0

    nc = bacc.Bacc(
        "TRN2", target_bir_lowering=False, debug=False, num_devices=num_devices
    )
    xf_h = nc.declare_dram_parameter("x", [C, T_], F32, isOutput=False)
    xbf_h = nc.declare_dram_parameter("xbf", [C, T_], BF16, isOutput=False)
    w1_h = nc.declare_dram_parameter("w1p", [C, 3, C], BF16, isOutput=False)
    w2_h = nc.declare_dram_parameter("w2p", [C, 3, C], BF16, isOutput=False)
    # pv rows: 0:a1/pi  1:c1  2:a2/pi  3:c2  4:b1  5:b2
    pv_h = nc.declare_dram_parameter("pv", [6, C], F32, isOutput=False)
    fd_h = nc.declare_dram_parameter("fdiag", [24, 128, 128], BF16, isOutput=False)
    out_h = nc.declare_dram_parameter("out", [C, T_], F32, isOutput=True)

    xf_r = xf_h.rearrange("(b p) t -> p b t", p=128)
    xbf_r = xbf_h.rearrange("(b p) t -> p b t", p=128)
    w1_r = w1_h.rearrange("(b p) d co -> p b (d co)", p=128)
    w2_r = w2_h.rearrange("(b p) d co -> p b (d co)", p=128)
    pv_r = pv_h.rearrange("r (b p) -> p r b", p=128)
    fd_r = fd_h.rearrange("k p m -> p k m")
    out_r = out_h.rearrange("(b p) t -> p b t", p=128)

    any_pe = any(e == "pe" for site in ep.values() for e in site)

    with TileContext(nc) as tc:
        with (
            tc.tile_pool(name="const", bufs=1) as cpool,
            tc.tile_pool(name="io", bufs=2) as iopool,
            tc.tile_pool(name="work", bufs=2) as wpool,
            tc.tile_pool(name="psum", bufs=4, space="PSUM") as ppool,
        ):
            w1sb = cpool.tile([128, NB, 3 * C], BF16, name="w1sb")
            nc.sync.dma_start(out=w1sb[:], in_=w1_r[:])
            w2sb = cpool.tile([128, NB, 3 * C], BF16, name="w2sb")
            nc.sync.dma_start(out=w2sb[:], in_=w2_r[:])
            pvsb = cpool.tile([128, 6, NB], F32, name="pvsb")
            nc.sync.dma_start(out=pvsb[:], in_=pv_r[:])
            negm = cpool.tile([128, 1], F32, name="negm")
            nc.vector.memset(negm[:], -MAGIC)
            if any_pe:
                fdsb = cpool.tile([128, 24, 128], BF16, name="fdsb")
                nc.sync.dma_start(out=fdsb[:], in_=fd_r[:])
            else:
                fdsb = None

            def fill_left(tile_, c1, c0):
                # fill cols [c0, c1) with the value at col c1 (log-doubling)
                rep_lo, have = c1, 1
                while rep_lo > c0:
                    w = min(have, rep_lo - c0)
                    nc.vector.tensor_copy(
                        tile_[:, :, rep_lo - w : rep_lo],
                        tile_[:, :, rep_lo : rep_lo + w],
                    )
                    rep_lo -= w
                    have += w

            def fill_right(tile_, c0, c1):
                # fill cols (c0, c1] ... i.e. [c0+1, c1+1) with value at col c0
                rep_hi, have = c0 + 1, 1
                while rep_hi < c1 + 1:
                    w = min(have, c1 + 1 - rep_hi)
                    nc.vector.tensor_copy(
                        tile_[:, :, rep_hi : rep_hi + w],
                        tile_[:, :, rep_hi - w : rep_hi],
                    )
                    rep_hi += w
                    have += w

            def mk_shadow(src, tag, lo, hi):
                """+1-shifted copy (sh[c] = src[c+1]) over src-read range
                [lo, hi) so odd-shift taps read 4B-aligned bf16.  On ScalarE:
                alignment-insensitive, spare capacity."""
                sh = wpool.tile([128, NB, W], BF16, tag=tag, bufs=1, name=tag)
                nc.scalar.activation(
                    out=sh[:, :, lo - 1 : hi - 1], in_=src[:, :, lo:hi],
                    func=IDENT, scale=1.0,
                )
                return sh

            def fir_taps(dst, srcs_shifts_taps, m, n, eng, fd_base, shadows=None):
                """dst[:, b, m:m+n] = sum taps[j]*src[:, b, m+shift_j : ...].

                srcs_shifts_taps: list of (src_tile, shift, tap).
                eng: per-block engine list. 'pe' uses diagonal matmuls with
                fdiag[fd_base + j] and evicts via ScalarE.  shadows: id(src)
                -> +1-shifted copy, used by DVE blocks for odd shifts.
                """
                shadows = shadows or {}
                blocks_by_eng = {}
                for b in range(NB):
                    blocks_by_eng.setdefault(eng[b], []).append(b)
                for e, blks in blocks_by_eng.items():
                    if e == "dve":
                        if blks == list(range(NB)):
                            sel = [slice(None)]
                        else:
                            sel = blks
                        for bsel in sel:
                            dv = dst[:, bsel, m : m + n]
                            first = True
                            for src, sh, tap in srcs_shifts_taps:
                                off = m + sh
                                if off % 2 and id(src) in shadows:
                                    src = shadows[id(src)]
                                    off -= 1
                                sv = src[:, bsel, off : off + n]
                                if first:
                                    nc.vector.tensor_scalar(
                                        out=dv, in0=sv, scalar1=tap, scalar2=None,
                                        op0=MULT,
                                    )
                                    first = False
                                elif DVE_TAP_FORM == "pair":
                                    tmp = wpool.tile(
                                        [128, NB, W], BF16, tag="ttmp", bufs=2,
                                        name="ttmp",
                                    )
                                    tv = tmp[:, bsel, m : m + n]
                                    nc.vector.tensor_scalar(
                                        out=tv, in0=sv, scalar1=tap, scalar2=None,
                                        op0=MULT,
                                    )
                                    nc.vector.tensor_tensor(
                                        out=dv, in0=tv, in1=dv, op=ADD
                                    )
                                else:
                                    nc.vector.scalar_tensor_tensor(
                                        out=dv, in0=sv, scalar=tap, in1=dv,
                                        op0=MULT, op1=ADD,
                                    )
                    elif e == "pe":
                        nt = len(srcs_shifts_taps)
                        for b in blks:
                            ps = ppool.tile([128, n], F32, tag="ps", bufs=4)
                            for o in range(0, n, 512):
                                sw = min(512, n - o)
                                for j, (src, sh, tap) in enumerate(srcs_shifts_taps):
                                    nc.tensor.matmul(
                                        ps[:, o : o + sw],
                                        fdsb[:, fd_base + j, :],
                                        src[:, b, m + sh + o : m + sh + o + sw],
                                        start=(j == 0),
                                        stop=(j == nt - 1),
                                    )
                            nc.scalar.activation(
                                out=dst[:, b, m : m + n], in_=ps[:],
                                func=IDENT, scale=1.0,
                            )
                    else:
                        raise ValueError(e)

            def fir_up(dst_e, dst_o, src, m, eng, fd_base=0):
                n = W - 2 * m
                shadows = {}
                if USE_SHADOWS and any(e == "dve" for e in eng):
                    shadows[id(src)] = mk_shadow(src, "shu", m - 3, W - m + 3)
                fir_taps(dst_e, [(src, a - 3, FE[a]) for a in range(6)], m, n, eng,
                         fd_base, shadows)
                fir_taps(dst_o, [(src, a - 2, FO[a]) for a in range(6)], m, n, eng,
                         fd_base + 6, shadows)

            def fir_down(dst, se, so, m, eng):
                n = W - 2 * m
                shadows = {}
                if USE_SHADOWS and any(e == "dve" for e in eng):
                    shadows[id(se)] = mk_shadow(se, "she", m - 3, W - m + 3)
                    shadows[id(so)] = mk_shadow(so, "sho", m - 3, W - m + 3)
                taps = [(so, b - 3, GO[b]) for b in range(6)] + [
                    (se, b - 2, GE[b]) for b in range(6)
                ]
                fir_taps(dst, taps, m, n, eng, 12, shadows)

            def snake(ye, yo, m, r_a, r_c):
                # s = y + c*sin^2(pi*g),  g = w - rint(w),  w = y*(a/pi)
                n = W - 2 * m
                for yp in (ye, yo):
                    g = wpool.tile([128, NB, W], BF16, tag="tcos", bufs=BUFS["tcos"], name="g")
                    for b in range(NB):
                        vt = wpool.tile([128, W], F32, tag="vt", bufs=4, name="vt")
                        t2 = wpool.tile([128, W], BF16, tag="t2", bufs=4, name="t2")
                        # v = rint(w) + M  (fused multiply-add, fp32 RNE)
                        nc.vector.tensor_scalar(
                            out=vt[:, m : m + n], in0=yp[:, b, m : m + n],
                            scalar1=pvsb[:, r_a, b : b + 1], scalar2=MAGIC,
                            op0=MULT, op1=ADD,
                        )
                        # t2 = rint(w) = v - M  (exact in bf16 for |w| < 64)
                        nc.scalar.activation(
                            out=t2[:, m : m + n], in_=vt[:, m : m + n],
                            func=IDENT, bias=negm[:, 0:1], scale=1.0,
                        )
                        # g = y*(a/pi) - rint(w)
                        nc.vector.scalar_tensor_tensor(
                            out=g[:, b, m : m + n], in0=yp[:, b, m : m + n],
                            scalar=pvsb[:, r_a, b : b + 1],
                            in1=t2[:, m : m + n], op0=MULT, op1=SUB,
                        )
                    # sn = sin(pi*g), in-place over g
                    nc.scalar.activation(
                        out=g[:, :, m : m + n], in_=g[:, :, m : m + n],
                        func=SIN, scale=PI,
                    )
                    # sq = sn*sn, in-place over g
                    sq_eng = nc.gpsimd if SQ_ENGINE == "gps" else nc.vector
                    sq_eng.tensor_tensor(
                        out=g[:, :, m : m + n], in0=g[:, :, m : m + n],
                        in1=g[:, :, m : m + n], op=MULT,
                    )
                    if MERGE_FORM == "stt":
                        for b in range(NB):
                            nc.vector.scalar_tensor_tensor(
                                out=yp[:, b, m : m + n], in0=g[:, b, m : m + n],
                                scalar=pvsb[:, r_c, b : b + 1],
                                in1=yp[:, b, m : m + n], op0=MULT, op1=ADD,
                            )
                    else:
                        for b in range(NB):
                            # q = sq*c (4x-eligible tensor_scalar)
                            nc.vector.tensor_scalar(
                                out=g[:, b, m : m + n], in0=g[:, b, m : m + n],
                                scalar1=pvsb[:, r_c, b : b + 1], scalar2=None,
                                op0=MULT,
                            )
                        # s = q + y, in-place over y
                        m_eng = nc.gpsimd if MERGE_FORM == "gps_tt" else nc.vector
                        m_eng.tensor_tensor(
                            out=yp[:, :, m : m + n], in0=g[:, :, m : m + n],
                            in1=yp[:, :, m : m + n], op=ADD,
                        )

            def s_clamp(ye, yo, first, last):
                # down reads so[t-3..t+2], se[t-2..t+3]; 2T-grid edge clamping
                if first:  # s[u<0] -> se[HALO]
                    for c in (HALO - 3, HALO - 2, HALO - 1):
                        nc.vector.tensor_copy(
                            yo[:, :, c : c + 1], ye[:, :, HALO : HALO + 1]
                        )
                    for c in (HALO - 2, HALO - 1):
                        nc.vector.tensor_copy(
                            ye[:, :, c : c + 1], ye[:, :, HALO : HALO + 1]
                        )
                if last:  # s[u>2T-1] -> so[W-HALO-1]
                    e = W - HALO - 1
                    for c in (e + 1, e + 2, e + 3):
                        nc.vector.tensor_copy(
                            ye[:, :, c : c + 1], yo[:, :, e : e + 1]
                        )
                        nc.vector.tensor_copy(
                            yo[:, :, c : c + 1], yo[:, :, e : e + 1]
                        )

            def conv(dst, src, wsb, r_bias, m):
                n = W - 2 * m
                for co in range(NB):
                    ps = ppool.tile([128, n], F32, tag="ps", bufs=4, name="cps")
                    for o in range(0, n, 512):
                        sw = min(512, n - o)
                        k = 0
                        for d in range(3):
                            for cb in range(NB):
                                nc.tensor.matmul(
                                    ps[:, o : o + sw],
                                    wsb[:, cb, d * C + co * 128 : d * C + (co + 1) * 128],
                                    src[:, cb, m + o + d - 1 : m + o + d - 1 + sw],
                                    start=(k == 0),
                                    stop=(k == 11),
                                )
                                k += 1
                    nc.scalar.activation(
                        out=dst[:, co, m : m + n], in_=ps[:],
                        func=IDENT, bias=pvsb[:, r_bias, co : co + 1],
                        scale=1.0,
                    )

            # --- stage-major emission over groups of chunks: each engine
            # gets GROUP chunks of the same stage back-to-back, so e.g. PE
            # runs chunk i+1's FIR matmuls while DVE/ACT/GpSimd run chunk
            # i's snake chain. ---
            def st_load(s):
                ci = s["ci"]
                lo, hi = ci * tc_ - HALO, ci * tc_ + tc_ + HALO
                slo, shi = max(lo, 0), min(hi, T_)
                d0 = slo - lo
                xbf = iopool.tile([128, NB, W], BF16, tag="xbf", bufs=BUFS["xbf"],
                                  name="xbf")
                nc.sync.dma_start(
                    out=xbf[:, :, d0 : d0 + shi - slo], in_=xbf_r[:, :, slo:shi]
                )
                if s["first"]:
                    fill_left(xbf, HALO, 0)
                if s["last"]:
                    fill_right(xbf, W - HALO - 1, W - 1)
                s["xbf"] = xbf

            def st_afa(k):
                up_m, z_m = ((M_UP1, M_Z1), (M_UP2, M_Z2), (M_UP3, M_Z3))[k]
                r_a, r_c = (0, 1) if k < 2 else (2, 3)

                def up(s):
                    src = s["xbf"] if k == 0 else s["zc"]
                    ye = wpool.tile([128, NB, W], BF16, tag="ye", bufs=BUFS["ye"],
                                    name="ye")
                    yo = wpool.tile([128, NB, W], BF16, tag="yo", bufs=BUFS["yo"],
                                    name="yo")
                    fir_up(ye, yo, src, up_m, ep[("u1", "u2", "u3")[k]])
                    s["ye"], s["yo"] = ye, yo

                def snk(s):
                    snake(s["ye"], s["yo"], up_m, r_a, r_c)
                    s_clamp(s["ye"], s["yo"], s["first"], s["last"])

                def down(s):
                    z = wpool.tile([128, NB, W], BF16, tag="z", bufs=BUFS["z"],
                                   name="z")
                    fir_down(z, s["ye"], s["yo"], z_m, ep[("d1", "d2", "d3")[k]])
                    if k < 2:
                        if s["first"]:
                            nc.vector.memset(z[:, :, HALO - 1 : HALO], 0.0)
                        if s["last"]:
                            nc.vector.memset(
                                z[:, :, W - HALO : W - HALO + 1], 0.0
                            )
                    s["z"] = z

                return [up, snk, down]

            def st_conv(k):
                def run(s):
                    wsb, r_bias, m = ((w1sb, 4, M_C1), (w2sb, 5, M_C2))[k]
                    zc = wpool.tile([128, NB, W], BF16, tag="zc", bufs=BUFS["zc"],
                                    name="zc")
                    conv(zc, s["z"], wsb, r_bias, m)
                    if s["first"]:
                        fill_left(zc, HALO, HALO - 3)
                    if s["last"]:
                        fill_right(zc, W - HALO - 1, W - HALO + 2)
                    s["zc"] = zc

                return run

            def st_resid(s):
                ci = s["ci"]
                t0 = ci * tc_
                xf = iopool.tile([128, NB, tc_], F32, tag="xf", bufs=BUFS["xf"],
                                 name="xf")
                nc.sync.dma_start(out=xf[:], in_=xf_r[:, :, t0 : t0 + tc_])
                nc.gpsimd.tensor_tensor(
                    out=xf[:], in0=s["z"][:, :, HALO : HALO + tc_], in1=xf[:],
                    op=ADD,
                )
                nc.sync.dma_start(out=out_r[:, :, t0 : t0 + tc_], in_=xf[:])

            stages = (
                [st_load]
                + st_afa(0)
                + [st_conv(0)]
                + st_afa(1)
                + [st_conv(1)]
                + st_afa(2)
                + [st_resid]
            )
            for g0 in range(0, nchunk, GROUP):
                chunks = [
                    {"ci": ci, "first": ci == 0, "last": ci == nchunk - 1}
                    for ci in range(g0, min(g0 + GROUP, nchunk))
                ]
                for stage in stages:
                    for s in chunks:
                        stage(s)

    nc.compile()
    return nc
def _host_inputs(x, w1, b1, w2, b2, alpha1, beta1, alpha2, beta2):
    a1 = np.exp(np.asarray(alpha1, np.float64))
    c1 = 1.0 / (np.exp(np.asarray(beta1, np.float64)) + EPS)
    a2 = np.exp(np.asarray(alpha2, np.float64))
    c2 = 1.0 / (np.exp(np.asarray(beta2, np.float64)) + EPS)
    pv = np.stack([a1 / PI, c1, a2 / PI, c2,
                   np.asarray(b1, np.float64), np.asarray(b2, np.float64)])
    pv = np.ascontiguousarray(pv).astype(np.float32)
    w1p = np.ascontiguousarray(np.asarray(w1).transpose(1, 2, 0)).astype(
        ml_dtypes.bfloat16
    )
    w2p = np.ascontiguousarray(np.asarray(w2).transpose(1, 2, 0)).astype(
        ml_dtypes.bfloat16
    )
    # diag filter tiles for the PE tap path: 6 FE, 6 FO, 6 GO, 6 GE
    eye = np.eye(128, dtype=np.float32)
    taps24 = FE + FO + GO + GE
    fdiag = np.ascontiguousarray(
        np.stack([t * eye for t in taps24])
    ).astype(ml_dtypes.bfloat16)
    xbf = np.asarray(x).astype(ml_dtypes.bfloat16)
    return pv, w1p, w2p, fdiag, xbf


_compiled = {}


def _get_compiled():
    if "nc" not in _compiled:
        _compiled["nc"] = build()
    return _compiled["nc"]


def make_in_maps(x, w1, b1, w2, b2, alpha1, beta1, alpha2, beta2):
    pv, w1p, w2p, fdiag, xbf = _host_inputs(
        x, w1, b1, w2, b2, alpha1, beta1, alpha2, beta2
    )
    B = x.shape[0]
    return [
        {
            "x": np.ascontiguousarray(np.asarray(x[i], np.float32)),
            "xbf": np.ascontiguousarray(xbf[i]),
            "w1p": w1p,
            "w2p": w2p,
            "pv": pv,
            "fdiag": fdiag,
        }
        for i in range(B)
    ]


def kernel(x, w1, b1, w2, b2, alpha1, beta1, alpha2, beta2):
    x = np.asarray(x)
    B = x.shape[0]
    assert x.shape == (B, C, T) and B == 8, x.shape
    nc = _get_compiled()
    in_maps = make_in_maps(x, w1, b1, w2, b2, alpha1, beta1, alpha2, beta2)
    res = run_bass_kernel_spmd(nc, in_maps, core_ids=list(range(B)))
    return np.stack([res.results[i]["out"] for i in range(B)], axis=0)
